# revision 2
# baseline (speedup 1.0000x reference)
"""Trainium2 Bass kernel v2 for the BiTRF dense transformer block.

Same math as baseline (see kernel.py docstring), restructured for the
TimelineSim cost model where DMA_ENGINES (360 B/ns serialized pipe) and
PE are the binding resources:

  - one activation table (natural_log_exp_and_others: Exp/Ln/Identity/Relu)
    for the whole kernel; LN rstd = Exp(-0.5*Ln(var+eps)) instead of
    Sqrt+Newton, so no mid-kernel table swaps.
  - DMA issue order sorted by need-time; h2o weight stream (e4m3 pairs of
    512-wide vocab tiles) self-clocked behind the FFN weights.
  - FFN2 + LN2 row-split (two 128-row halves) so stage E's first matmul
    issues right after the last FFN2 matmul with no PE bubble.
  - stage E: vocab processed in 1024-wide pair units; rt0 leads rt1 by K
    units (bounded W-tile reuse window).  exp+accum on ACT over 1024-wide
    pairs; PSUM->SBUF logits eviction on DVE (some pairs on ACT); the
    log_softmax subtract runs in-place on L16 (DVE 4x f16 mode, a few
    chunks on Pool), overlapped under the other row-tile's matmuls; output
    DMAs issue from the ACT HWDGE queue so their waits never block the SP
    weight stream.
"""

import contextlib
import math
import os

import ml_dtypes
import numpy as np

import functools

import concourse.mybir as mybir
import concourse.tile as tile
from concourse import bacc
from concourse.bass_utils import run_bass_kernel_spmd
from concourse.masks import make_identity

# Steer bacc's activation-table selection to the joint Exp+Ln table so the
# kernel runs on ONE table (no mid-kernel InstLoadActFuncSet swaps).  The
# act_func_set_id is the POSITION in act_info.json's list, so positions are
# preserved — only the non-joint tables are made unattractive for the funcs
# we use.  The chosen table genuinely contains every function we emit.
_AF = mybir.ActivationFunctionType
_OURS = {_AF.Exp, _AF.Ln, _AF.Identity, _AF.Relu, _AF.Copy}
_JOINT = "natural_log_exp_and_others"
_orig_get_tables = bacc.get_activation_tables


@functools.cache
def _patched_tables(arch):
    tabs = dict(_orig_get_tables(arch))
    if _JOINT in tabs and _OURS <= set(tabs[_JOINT]):
        tabs = {k: (v if k == _JOINT else set(v) - _OURS)
                for k, v in tabs.items()}
    return tabs


bacc.get_activation_tables = _patched_tables

B, L, D, H, DV, HID, V = 2, 1024, 1024, 16, 64, 4096, 32000
NCORES = 8
IC = L // NCORES        # 128 query rows per core
ROWS = B * IC           # 256 row-instances per core
P = 128
DC = D // P             # 8 feature chunks
HC = HID // P           # 32 hidden chunks
EPS = 1e-5

F32 = mybir.dt.float32
F32R = mybir.dt.float32r
BF16 = mybir.dt.bfloat16
F16 = mybir.dt.float16
E4 = mybir.dt.float8e4
AF = mybir.ActivationFunctionType
ALU = mybir.AluOpType
AX = mybir.AxisListType
DR = mybir.MatmulPerfMode.DoubleRow

# h2o vocab pair-units: 31 pairs of (512,512) + one odd 256 tile
UNITS = 32
UNIT_W = [1024] * 31 + [256]
KLEAD = 7               # rt0 leads rt1 by KLEAD units (W reuse window)
WBUFS = 9               # h2o weight-pair prefetch depth (8KB/partition each)


def _r(ap):
    return ap.bitcast(F32R)


def _softplus(x):
    return np.logaddexp(0.0, x.astype(np.float64))


def _build(p0, sp1, sp2, p3, bias_on, sc):
    """bias_on: {'h2o','fc','b1','b2'} — zero-bias paths are skipped.
    sc: immediates {inv_h2o, kH (FFN1 relu descale-to-8H), } — the FFN2
    descale is folded into the residual copy of Y (LN2 is scale-invariant).
    """
    inv_h2o = sc["inv_h2o"]
    hkeys = {(float(p0[h]), float(sp1[h]), float(sp2[h]), float(p3[h]))
             for h in range(H)}
    assert len(hkeys) == 1, "reassociated attention needs identical heads"

    nc = bacc.Bacc(None, target_bir_lowering=False, debug=False,
                   num_devices=NCORES)

    def inp(name, shape, dtype):
        return nc.dram_tensor(name, shape, dtype, kind="ExternalInput")

    x_rm = inp("x_rm", [P, 8, B, D], BF16)       # [jp, jc, b, feat]
    fcM = inp("fcM", [D, D], BF16)               # wv.T @ fc_w.T (host f64)
    w1T = inp("w1T", [D, HID], E4)               # x sw1 on host
    w2T = inp("w2T", [HID, D], BF16)
    h2oT = inp("h2oT", [D, V], E4)               # pre-scaled by sw on host
    if bias_on["fc"]:
        fcb2 = inp("fcb2", [P, DC], F32)         # fc_b + fc_w @ bv
    if bias_on["b1"]:
        b12 = inp("b12", [P, HC], F32)
    if bias_on["b2"]:
        b22 = inp("b22", [P, DC], F32)
    if bias_on["h2o"]:
        h2ob = inp("h2ob", [1, V], BF16)
        onesr = inp("onesr", [1, ROWS], BF16)
    onesc = inp("onesc", [P, 2], F32R)
    onesb = inp("onesb", [P, 2], BF16)
    onesp = inp("onesp", [1, P], F32R)
    ln1g = inp("ln1g", [P, DC], F32)
    ln1b = inp("ln1b", [P, DC], F32)
    ln2g8 = inp("ln2g8", [P, DC], F32)
    ln2b8 = inp("ln2b8", [P, DC], F32)
    u_in = inp("u_in", [P, 8, IC], BF16)         # [jp, jc, i]
    rsb_in = inp("rsb_in", [P, ROWS], F32)
    y = nc.dram_tensor("y", [ROWS, V], F16, kind="ExternalOutput")
    DBG = bool(os.environ.get("K2_DEBUG"))
    if DBG:
        dYhi = nc.dram_tensor("dYhi", [P, DC, ROWS], E4, kind="ExternalOutput")
        dYlo = nc.dram_tensor("dYlo", [P, DC, ROWS], E4, kind="ExternalOutput")
        dYsb = nc.dram_tensor("dYsb", [P, DC, ROWS], BF16, kind="ExternalOutput")
        dH = nc.dram_tensor("dH", [P, HC, ROWS], BF16, kind="ExternalOutput")
        dFF = nc.dram_tensor("dFF", [P, DC, ROWS], F32, kind="ExternalOutput")
        dZhi = nc.dram_tensor("dZhi", [P, DC, ROWS], E4, kind="ExternalOutput")
        dZlo = nc.dram_tensor("dZlo", [P, DC, ROWS], E4, kind="ExternalOutput")

    with tile.TileContext(nc) as tc, contextlib.ExitStack() as top:
        c0 = top.enter_context(tc.tile_pool(name="const0", bufs=1))
        wp = top.enter_context(tc.tile_pool(name="h2o_w", bufs=WBUFS))
        zp = top.enter_context(tc.tile_pool(name="zmid", bufs=1))

        # ---- DMA issue order = need order (single SP HWDGE pipe) ----
        # attn-lifetime tensors on the RIGHT SBUF stack: freed pre-FFN,
        # independently of the left-stack pool nesting.
        xstk2 = contextlib.ExitStack()   # fcM/rsB: freed after fc
        xp2 = xstk2.enter_context(tc.tile_pool(name="xpool2", bufs=1,
                                               side="right"))
        xstk = contextlib.ExitStack()    # u/x/identb: freed after attention
        xp0 = xstk.enter_context(tc.tile_pool(name="xpool", bufs=1,
                                              side="right"))
        u_sb = xp0.tile([P, 8, IC], BF16, name="u_sb")
        nc.sync.dma_start(u_sb[:], u_in.ap())
        x_sb = xp0.tile([P, 8, B, D], BF16, name="x_sb")
        for jc in range(8):     # half-feature granularity: earlier starts
            for hf in range(2):
                nc.sync.dma_start(x_sb[:, jc, :, hf * 512:(hf + 1) * 512],
                                  x_rm.ap()[:, jc, :, hf * 512:(hf + 1) * 512])
        rsB = xp2.tile([P, ROWS], F32, name="rsB")
        nc.sync.dma_start(rsB[:], rsb_in.ap())
        ones_col = c0.tile([P, 2], F32R, name="ones_col")
        nc.sync.dma_start(ones_col[:], onesc.ap())
        ones_colb = c0.tile([P, 2], BF16, name="ones_colb")
        nc.sync.dma_start(ones_colb[:], onesb.ap())
        ones_rowP = c0.tile([1, P], F32R, name="ones_rowP")
        nc.sync.dma_start(ones_rowP[:], onesp.ap())
        if bias_on["fc"]:
            fcb_sb = c0.tile([P, DC], F32, name="fcb_sb")
            nc.sync.dma_start(fcb_sb[:], fcb2.ap())
        g1_sb = c0.tile([P, DC], F32, name="g1_sb")
        nc.sync.dma_start(g1_sb[:], ln1g.ap())
        b1g_sb = c0.tile([P, DC], F32, name="b1g_sb")
        nc.sync.dma_start(b1g_sb[:], ln1b.ap())
        if bias_on["b1"]:
            b1_sb = c0.tile([P, HC], F32, name="b1_sb")
            nc.sync.dma_start(b1_sb[:], b12.ap())
        # fc merged matrix, split in halves so fc can start on dc 0-3 early
        fcM_sb = xp2.tile([P, DC, D], BF16, name="fcM_sb")
        fcM_t = fcM.ap().rearrange("(c p) f -> p c f", p=P)
        nc.sync.dma_start(fcM_sb[:, 0:4], fcM_t[:, 0:4])
        nc.sync.dma_start(fcM_sb[:, 4:8], fcM_t[:, 4:8])
        identb = xp0.tile([P, P], BF16, name="identb")
        make_identity(nc, identb[:])
        # pin the one activation table (Ln+Exp+Identity+Relu) at t~0
        warm = c0.tile([P, 2], F32, name="warm")
        nc.any.memset(warm[:], 1.0)
        nc.scalar.activation(warm[:], warm[:], AF.Ln)
        nc.scalar.activation(warm[:], warm[:], AF.Exp)

        Zhi = zp.tile([P, DC, ROWS], E4, name="Zhi")
        Zlo = zp.tile([P, DC, ROWS], E4, name="Zlo")
        parts = c0.tile([P, B, UNITS], F32, name="parts")
        L16 = [None, None]      # per-rt logits staging, opened in phases

        with contextlib.ExitStack() as s1:
            fcp0 = s1.enter_context(tc.tile_pool(name="fcc", bufs=1))
            # FFN weight pools nest inside s1 (close before stage E frees
            # SBUF for the rt1 logits half).  W1 streamed (each hs used
            # once); W2 ds-major (reused across row halves).
            w1p = s1.enter_context(tc.tile_pool(name="w1pool", bufs=4))
            w1T_t = w1T.ap().rearrange("(c p) m -> p c m", p=P)
            W1s = []
            for hs in range(8):
                W1t = w1p.tile([P, DC, 512], E4, name="W1t")
                nc.sync.dma_start(W1t[:],
                                  w1T_t[:, :, hs * 512:(hs + 1) * 512])
                W1s.append(W1t)
            if bias_on["b2"]:
                b2_sb = c0.tile([P, DC], F32, name="b2_sb")
                nc.sync.dma_start(b2_sb[:], b22.ap())
            g8_sb = c0.tile([P, DC], F32, name="g8_sb")
            nc.sync.dma_start(g8_sb[:], ln2g8.ap())
            b8_sb = c0.tile([P, DC], F32, name="b8_sb")
            nc.sync.dma_start(b8_sb[:], ln2b8.ap())

            # -------- stage AB: attention + fc (merged via fcM) --------
            with contextlib.ExitStack() as sb:
                up = sb.enter_context(tc.tile_pool(name="attn_u", bufs=2))
                ap_ = sb.enter_context(tc.tile_pool(name="attn_a", bufs=1))
                F1 = ap_.tile([P, DC, ROWS], F32R, name="F1")

                with contextlib.ExitStack() as sba:
                    pp = sba.enter_context(tc.tile_pool(name="attn_p",
                                                        bufs=1, space="PSUM"))
                    pxas = [pp.tile([P, 512], F32, name=f"pxa{q}", bufs=1)
                            for q in range(2)]
                    xarm = ap_.tile([P, B, D], BF16, name="xarm")
                    xaT = ap_.tile([P, DC, ROWS], BF16, name="xaT")

                    def transpose_one(b, fc_):
                        pt = pp.tile([P, P], BF16, name="pt", bufs=2)
                        nc.tensor.transpose(
                            pt[:], xarm[:, b, fc_ * P:(fc_ + 1) * P],
                            identb[:])
                        nc.scalar.activation(
                            xaT[:, fc_, b * IC:(b + 1) * IC], pt[:],
                            AF.Identity)

                    # b0's transposes slot into pxa(b1)'s x-DMA wait gaps
                    for b in range(B):
                        for jc in range(8):
                            for half in range(2):
                                nc.tensor.matmul(
                                    pxas[half][:],
                                    u_sb[:, jc, :],
                                    x_sb[:, jc, b,
                                         half * 512:(half + 1) * 512],
                                    start=(jc == 0), stop=(jc == 7))
                            if b == 1:
                                transpose_one(0, jc)
                        for half in range(2):
                            nc.scalar.activation(
                                xarm[:, b, half * 512:(half + 1) * 512],
                                pxas[half][:], AF.Identity)
                    for fc_ in range(DC):
                        transpose_one(1, fc_)
                    xstk.close()   # u/x/identb SBUF freed

                # F[dout, rows] = fcM.T @ xa^T; LN1 sums interleave per do.
                # pv/psB share one 2-bank rotation; the two LN1 sum rows
                # pack into a single bank at partition offsets 0/32.
                pp2 = sb.enter_context(tc.tile_pool(name="fc_p", bufs=1,
                                                    space="PSUM"))
                ps12 = pp2.tile([2, 2, ROWS], F32, name="ps12")
                ps_sum, ps_sq = ps12[:, 0, :], ps12[:, 1, :]
                for do in range(DC):
                    pv = pp2.tile([P, ROWS], F32, name="pv", bufs=2)
                    for dc in range(DC):
                        nc.tensor.matmul(
                            pv[:],
                            fcM_sb[:, dc, do * P:(do + 1) * P],
                            xaT[:, dc, :],
                            start=(dc == 0), stop=(dc == DC - 1))
                    if bias_on["fc"]:
                        t = up.tile([P, ROWS], F32, name="fvt", bufs=3)
                        nc.vector.tensor_mul(t[:], pv[:], rsB[:])
                        nc.vector.tensor_scalar(F1[:, do], t[:],
                                                fcb_sb[:, do:do + 1], None,
                                                ALU.add)
                    else:
                        nc.vector.tensor_mul(F1[:, do], pv[:], rsB[:])
                    sq = up.tile([P, ROWS], F32R, name="sqr", bufs=2)
                    nc.vector.tensor_mul(sq[:], F1[:, do], F1[:, do])
                    # one accumulation group for both halves of the bank:
                    # start zeroes the WHOLE bank, so only the very first
                    # matmul may set it (and only the very last stops)
                    nc.tensor.matmul(ps12[:, 0, :], ones_col[:], F1[:, do],
                                     start=(do == 0), stop=False,
                                     skip_group_check=True)
                    nc.tensor.matmul(ps12[:, 1, :], ones_col[:], sq[:],
                                     start=False, stop=(do == DC - 1),
                                     skip_group_check=True)

                # ---------------- LN1 (full-row, no-sqrt chain) --------
                lp = sb.enter_context(tc.tile_pool(name="ln1c", bufs=1))
                mean = lp.tile([1, ROWS], F32, name="mean1")
                nc.vector.tensor_scalar(mean[:], ps12[0:1, 0, :], 1.0 / D,
                                        None, ALU.mult)
                ex2 = lp.tile([1, ROWS], F32, name="ex21")
                nc.vector.tensor_scalar(ex2[:], ps12[0:1, 1, :], 1.0 / D,
                                        None, ALU.mult)
                var = lp.tile([1, ROWS], F32, name="var1")
                nc.vector.tensor_mul(var[:], mean[:], mean[:])
                nc.vector.tensor_sub(var[:], ex2[:], var[:])
                veps = lp.tile([1, ROWS], F32, name="veps1")
                nc.vector.tensor_scalar(veps[:], var[:], EPS, None, ALU.add)
                lnv = lp.tile([1, ROWS], F32, name="lnv1")
                nc.scalar.activation(lnv[:], veps[:], AF.Ln)
                rstd0 = lp.tile([1, ROWS], F32, name="rstd1a")
                nc.scalar.activation(rstd0[:], lnv[:], AF.Exp, scale=-0.5)
                # one Newton step: y *= 1.5 - 0.5*v*y^2  (table err ^2)
                yn = lp.tile([1, ROWS], F32, name="yn1")
                nc.vector.tensor_mul(yn[:], rstd0[:], rstd0[:])
                nc.vector.tensor_mul(yn[:], yn[:], veps[:])
                nc.vector.tensor_scalar(yn[:], yn[:], -0.5, 1.5,
                                        ALU.mult, ALU.add)
                rstd = lp.tile([1, ROWS], F32, name="rstd1")
                nc.vector.tensor_mul(rstd[:], rstd0[:], yn[:])
                meanr = lp.tile([1, ROWS], F32R, name="meanr1")
                nc.vector.tensor_copy(meanr[:], mean[:])
                rstdr = lp.tile([1, ROWS], F32R, name="rstdr1")
                nc.vector.tensor_copy(rstdr[:], rstd[:])
                psB = pp2.tile([P, ROWS], F32, name="pv", bufs=2)
                nc.tensor.matmul(psB[:], ones_rowP[:], meanr[:],
                                 start=True, stop=True)
                meanB = lp.tile([P, ROWS], F32, name="meanB1")
                nc.vector.tensor_copy(meanB[:], psB[:])
                psB2 = pp2.tile([P, ROWS], F32, name="pv", bufs=2)
                nc.tensor.matmul(psB2[:], ones_rowP[:], rstdr[:],
                                 start=True, stop=True)
                rstdB = lp.tile([P, ROWS], F32, name="rstdB1")
                nc.vector.tensor_copy(rstdB[:], psB2[:])

                # t16 = 16*LN1(F) (16x folded into g/b on host); Yhi/Ylo
                # e4m3 split feeds the fp8 FFN1; the residual copy Y_sb is
                # pre-scaled by c=8*sw2 (folded FFN2 descale — LN2 is
                # scale-invariant)
                Y_sb = fcp0.tile([P, DC, ROWS], BF16, name="Y_sb")
                Yhi = fcp0.tile([P, DC, ROWS], E4, name="Yhi")
                Ylo = fcp0.tile([P, DC, ROWS], E4, name="Ylo")
                t16s = []
                for dc in range(DC):
                    eng = nc.gpsimd if dc in (3, 7) else nc.vector
                    t1 = lp.tile([P, ROWS], F32, name="t1_1", bufs=3)
                    eng.tensor_sub(t1[:], F1[:, dc], meanB[:])
                    t16 = lp.tile([P, ROWS], F32, name="t16", bufs=8)
                    eng.tensor_mul(t16[:], t1[:], rstdB[:])
                    eng.tensor_scalar(t16[:], t16[:],
                                      g1_sb[:, dc:dc + 1],
                                      b1g_sb[:, dc:dc + 1],
                                      ALU.mult, ALU.add)
                    nc.scalar.activation(Yhi[:, dc], t16[:], AF.Identity)
                    t16s.append(t16)
                for dc in range(DC):
                    nc.vector.tensor_sub(Ylo[:, dc], t16s[dc][:],
                                         Yhi[:, dc])
                    nc.vector.tensor_scalar(Y_sb[:, dc], t16s[dc][:],
                                            sc["cY16"], None, ALU.mult)

            xstk2.close()   # fcM/rsB freed

            # FFN2 weights (ds-major; last two loads ride the idle Pool
            # SWDGE queue so their buffer-release waits cannot block the
            # SP stream) followed by the self-clocked h2o pair stream.
            w2p = s1.enter_context(tc.tile_pool(name="w2pool", bufs=8))
            w2T_t = w2T.ap().rearrange("(c p) m -> p c m", p=P)
            W2s = [None] * 8
            for ds2 in range(2):
                for g in range(4):
                    W2t = w2p.tile([P, 8, 512], BF16, name="W2t")
                    nc.sync.dma_start(
                        W2t[:], w2T_t[:, g * 8:(g + 1) * 8,
                                      ds2 * 512:(ds2 + 1) * 512])
                    W2s[ds2 * 4 + g] = W2t

            h2oT_t = h2oT.ap().rearrange("(c p) v -> p c v", p=P)
            W_pairs = []
            for pu in range(UNITS):
                w = UNIT_W[pu]
                W_sb = wp.tile([P, DC, 1024], E4, name="W_sb")
                nc.sync.dma_start(W_sb[:, :, :w],
                                  h2oT_t[:, :, pu * 1024:pu * 1024 + w])
                W_pairs.append(W_sb)
            if bias_on["h2o"]:
                bias_sb = c0.tile([1, V], BF16, name="bias_sb")
                nc.sync.dma_start(bias_sb[:], h2ob.ap())
                ones_row = c0.tile([1, ROWS], BF16, name="ones_row_z")
                nc.sync.dma_start(ones_row[:], onesr.ap())

            # ---------------- stage D: FFN (bf16) ----------------
            # E-stage rt0 psum pool opens FIRST (right after the attention
            # pools free) so it gets bank-aligned space disjoint from the
            # FFN/LN2 psums — rt0's stage-E matmuls overlap LN2(r1) at
            # runtime.  Entered on `top` so it survives into stage E.
            pe0 = top.enter_context(tc.tile_pool(name="eps0", bufs=2,
                                                 space="PSUM"))
            with contextlib.ExitStack() as sd:
                hp = sd.enter_context(tc.tile_pool(name="hpool", bufs=1))
                ffp = sd.enter_context(tc.tile_pool(name="ffpool", bufs=1,
                                                    side="right"))
                pd = sd.enter_context(tc.tile_pool(name="psD", bufs=2,
                                                   space="PSUM"))
                H_sb = hp.tile([P, HC, ROWS], BF16, name="H_sb")
                for hs in range(8):           # 512-wide hid slices
                    for m2 in range(4):       # 128-wide subchunks
                        psh = pd.tile([P, ROWS], F32, name="psh", bufs=2)
                        for dp in range(4):
                            nc.tensor.matmul(
                                psh[:],
                                W1s[hs][:, 2 * dp:2 * dp + 2,
                                        m2 * P:(m2 + 1) * P],
                                Yhi[:, 2 * dp:2 * dp + 2, :],
                                start=(dp == 0), stop=False, perf_mode=DR)
                        for dp in range(4):
                            nc.tensor.matmul(
                                psh[:],
                                W1s[hs][:, 2 * dp:2 * dp + 2,
                                        m2 * P:(m2 + 1) * P],
                                Ylo[:, 2 * dp:2 * dp + 2, :],
                                start=False, stop=(dp == 3), perf_mode=DR)
                        hcix = hs * 4 + m2
                        # H = relu(mid) bf16 = Relu(kH*psh); alternate
                        # ACT/DVE so neither engine paces FFN1
                        if hcix % 2 == 0 and not bias_on["b1"]:
                            nc.vector.tensor_scalar(H_sb[:, hcix], psh[:],
                                                    sc["kH"], 0.0,
                                                    ALU.mult, ALU.max)
                        else:
                            bkw = dict(scale=sc["kH"])
                            if bias_on["b1"]:
                                bkw["bias"] = b1_sb[:, hcix:hcix + 1]
                            nc.scalar.activation(H_sb[:, hcix], psh[:],
                                                 AF.Relu, **bkw)

                # FFN2 ds-outer, row-split inner + per-half LN2 (bf16 sums)
                FF = ffp.tile([P, DC, ROWS], F32R, name="FF")
                lp2 = sd.enter_context(tc.tile_pool(name="ln2c", bufs=1))
                pl2 = sd.enter_context(tc.tile_pool(name="ln2p", bufs=1,
                                                    space="PSUM"))
                ps2c = [pl2.tile([2, 2, IC], F32, name=f"ps2c_{r}")
                        for r in range(B)]

                def emit_ln2(r):
                    rs = slice(r * IC, (r + 1) * IC)
                    # LN2 half-row chain (no sqrt) + fp8 hi/lo split
                    mean = lp2.tile([1, IC], F32, name="mean2")
                    nc.vector.tensor_scalar(mean[:], ps2c[r][0:1, 0, :],
                                            1.0 / D, None, ALU.mult)
                    ex2 = lp2.tile([1, IC], F32, name="ex22")
                    nc.vector.tensor_scalar(ex2[:], ps2c[r][0:1, 1, :],
                                            1.0 / D, None, ALU.mult)
                    var = lp2.tile([1, IC], F32, name="var2")
                    nc.vector.tensor_mul(var[:], mean[:], mean[:])
                    nc.vector.tensor_sub(var[:], ex2[:], var[:])
                    veps = lp2.tile([1, IC], F32, name="veps2")
                    nc.vector.tensor_scalar(veps[:], var[:], EPS, None,
                                            ALU.add)
                    lnv = lp2.tile([1, IC], F32, name="lnv2")
                    nc.scalar.activation(lnv[:], veps[:], AF.Ln)
                    rstd0 = lp2.tile([1, IC], F32, name="rstd2a")
                    nc.scalar.activation(rstd0[:], lnv[:], AF.Exp, scale=-0.5)
                    yn = lp2.tile([1, IC], F32, name="yn2")
                    nc.vector.tensor_mul(yn[:], rstd0[:], rstd0[:])
                    nc.vector.tensor_mul(yn[:], yn[:], veps[:])
                    nc.vector.tensor_scalar(yn[:], yn[:], -0.5, 1.5,
                                            ALU.mult, ALU.add)
                    rstd = lp2.tile([1, IC], F32, name="rstd2")
                    nc.vector.tensor_mul(rstd[:], rstd0[:], yn[:])
                    meanr = lp2.tile([1, IC], F32R, name="meanr2")
                    nc.vector.tensor_copy(meanr[:], mean[:])
                    rstdr = lp2.tile([1, IC], F32R, name="rstdr2")
                    nc.vector.tensor_copy(rstdr[:], rstd[:])
                    psB = pd.tile([P, ROWS], F32, name="psh", bufs=2)
                    nc.tensor.matmul(psB[:, 0:IC], ones_rowP[:], meanr[:],
                                     start=True, stop=True)
                    meanB = lp2.tile([P, IC], F32, name="meanB2")
                    nc.vector.tensor_copy(meanB[:], psB[:, 0:IC])
                    psB2 = pd.tile([P, ROWS], F32, name="psh", bufs=2)
                    nc.tensor.matmul(psB2[:, 0:IC], ones_rowP[:], rstdr[:],
                                     start=True, stop=True)
                    rstdB = lp2.tile([P, IC], F32, name="rstdB2")
                    nc.vector.tensor_copy(rstdB[:], psB2[:, 0:IC])
                    t8s = []
                    for dc in range(DC):
                        t8 = lp2.tile([P, IC], F32, name="t82", bufs=8)
                        nc.vector.tensor_sub(t8[:], FF[:, dc, rs], meanB[:])
                        nc.vector.tensor_mul(t8[:], t8[:], rstdB[:])
                        nc.vector.tensor_scalar(t8[:], t8[:],
                                                g8_sb[:, dc:dc + 1],
                                                b8_sb[:, dc:dc + 1],
                                                ALU.mult, ALU.add)
                        nc.scalar.activation(Zhi[:, dc, rs], t8[:],
                                             AF.Identity)
                        t8s.append(t8)
                    for dc in range(DC):
                        nc.vector.tensor_sub(Zlo[:, dc, rs], t8s[dc][:],
                                             Zhi[:, dc, rs])

                for ds2 in range(2):
                    for r in range(B):
                        rs = slice(r * IC, (r + 1) * IC)
                        for m2 in range(4):
                            do = ds2 * 4 + m2
                            pswt = pd.tile([P, ROWS], F32, name="psh",
                                           bufs=2)
                            psw = pswt[:, 0:IC]
                            for hc in range(HC):
                                nc.tensor.matmul(
                                    psw,
                                    W2s[ds2 * 4 + hc // 8][
                                        :, hc % 8, m2 * P:(m2 + 1) * P],
                                    H_sb[:, hc, rs],
                                    start=(hc == 0), stop=(hc == HC - 1))
                            if bias_on["b2"]:
                                tw = lp2.tile([P, IC], F32, name="tw",
                                              bufs=3)
                                nc.vector.tensor_scalar(tw[:], psw,
                                                        b2_sb[:, do:do + 1],
                                                        None, ALU.add)
                                nc.vector.tensor_add(FF[:, do, rs], tw[:],
                                                     Y_sb[:, do, rs])
                            else:
                                nc.vector.tensor_add(FF[:, do, rs], psw,
                                                     Y_sb[:, do, rs])
                            ffb = lp2.tile([P, IC], BF16, name="ffb", bufs=2)
                            nc.vector.tensor_copy(ffb[:], FF[:, do, rs])
                            sqb = lp2.tile([P, IC], BF16, name="sqb", bufs=2)
                            nc.vector.tensor_mul(sqb[:], ffb[:], ffb[:])
                            nc.tensor.matmul(ps2c[r][:, 0, :], ones_colb[:],
                                             ffb[:],
                                             start=(do == 0), stop=False,
                                             skip_group_check=True)
                            nc.tensor.matmul(ps2c[r][:, 1, :], ones_colb[:],
                                             sqb[:],
                                             start=False,
                                             stop=(do == DC - 1),
                                             skip_group_check=True)
                        if ds2 == 1:
                            # LN2(r) overlaps psw of the next row half /
                            # the first stage-E rt0 units
                            emit_ln2(r)
                if DBG:
                    nc.sync.dma_start(dYhi.ap(), Yhi[:])
                    nc.sync.dma_start(dYlo.ap(), Ylo[:])
                    nc.sync.dma_start(dYsb.ap(), Y_sb[:])
                    nc.sync.dma_start(dH.ap(), H_sb[:])
                    nc.sync.dma_start(dFF.ap(), FF[:].bitcast(F32))
                    nc.sync.dma_start(dZhi.ap(), Zhi[:])
                    nc.sync.dma_start(dZlo.ap(), Zlo[:])

        # ---------------- stage E: h2o + log_softmax ----------------
        with contextlib.ExitStack() as se:
            ep = se.enter_context(tc.tile_pool(name="h2o_e", bufs=2))
            pe1 = se.enter_context(tc.tile_pool(name="eps1", bufs=2,
                                                space="PSUM"))
            lsep = se.enter_context(tc.tile_pool(name="lsep", bufs=1))
            l16_p = se.enter_context(tc.tile_pool(name="l16", bufs=1,
                                                  side="right"))
            L16[0] = l16_p.tile([P, V], F16, name="L16a")
            L16[1] = l16_p.tile([P, V], F16, name="L16b")

            lse_t = [None, None]

            def emit_unit(rt, pu, evict_act):
                w = UNIT_W[pu]
                vs = pu * 1024
                rs = slice(rt * IC, (rt + 1) * IC)
                pool = pe0 if rt == 0 else pe1
                ps = pool.tile([P, 2, 512], F32, name=f"eps_t{rt}")
                W_sb = W_pairs[pu]
                ntile = 2 if w == 1024 else 1
                for j in range(ntile):
                    tw = 512 if w == 1024 else w
                    for g in range(4):
                        nc.tensor.matmul(
                            ps[:, j, :tw],
                            Zhi[:, 2 * g:2 * g + 2, rs],
                            W_sb[:, 2 * g:2 * g + 2,
                                 j * 512:j * 512 + tw],
                            start=(g == 0), stop=False, perf_mode=DR)
                    for g in range(4):
                        nc.tensor.matmul(
                            ps[:, j, :tw],
                            Zlo[:, 2 * g:2 * g + 2, rs],
                            W_sb[:, 2 * g:2 * g + 2,
                                 j * 512:j * 512 + tw],
                            start=False,
                            stop=(g == 3 and not bias_on["h2o"]),
                            perf_mode=DR)
                    if bias_on["h2o"]:
                        nc.tensor.matmul(
                            ps[:, j, :tw],
                            ones_row[:, rs],
                            bias_sb[:, vs + j * 512:vs + j * 512 + tw],
                            start=False, stop=True)
                src = ps[:, :, :] if w == 1024 else ps[:, 0, :w]
                sc = ep.tile([P, 1024], F16, name="esc")
                nc.scalar.activation(sc[:, :w], src, AF.Exp, scale=inv_h2o,
                                     accum_out=parts[:, rt, pu:pu + 1])
                dst = L16[rt][:, vs:vs + w]
                if evict_act:
                    nc.scalar.activation(dst, src, AF.Identity,
                                         scale=inv_h2o)
                else:
                    nc.vector.tensor_scalar(dst, src, inv_h2o, None,
                                            ALU.mult)

            def emit_lse(rt):
                s_t = lsep.tile([P, 1], F32, name=f"s_t{rt}")
                nc.vector.reduce_sum(s_t[:], parts[:, rt, :], axis=AX.X)
                lse = lsep.tile([P, 1], F32, name=f"lse{rt}")
                nc.scalar.activation(lse[:], s_t[:], AF.Ln)
                lse_t[rt] = lse

            # post-work: 8 in-place subtract chunks + 8 output DMAs per rt
            OGRP = ([(0, 1024), (1024, 3072)] +
                    [(g * 4096, 4096) for g in range(1, 7)] +
                    [(28672, 3328)])

            def post_ops(rt):
                ops = []
                for gi, (g0, gw) in enumerate(OGRP):
                    def sub_op(rt=rt, g0=g0, gw=gw, gi=gi):
                        sl = L16[rt][:, g0:g0 + gw]
                        if gi in (3, 5):      # middle chunks on idle Pool
                            nc.gpsimd.tensor_scalar(sl, sl, lse_t[rt][:],
                                                    None, ALU.subtract)
                        else:
                            nc.vector.tensor_scalar(sl, sl, lse_t[rt][:],
                                                    None, ALU.subtract)
                    def dma_op(rt=rt, g0=g0, gw=gw):
                        nc.scalar.dma_start(
                            y.ap()[rt * IC:(rt + 1) * IC, g0:g0 + gw],
                            L16[rt][:, g0:g0 + gw])
                    ops.append(sub_op)
                    ops.append(dma_op)
                return ops

            # mid-phase: evictions all-DVE (ACT is exp-bound); in the
            # rt1-only tail slots alternate evictions DVE/ACT.
            pending = []
            for s in range(UNITS + KLEAD):
                if s < UNITS:
                    emit_unit(0, s, evict_act=False)
                    if s == UNITS - 1:
                        emit_lse(0)
                        pending = post_ops(0)
                if s >= KLEAD:
                    pu = s - KLEAD
                    emit_unit(1, pu, evict_act=(s >= UNITS and pu % 2 == 0))
                for _ in range(3):
                    if pending:
                        pending.pop(0)()
            while pending:
                pending.pop(0)()
            emit_lse(1)
            for op in post_ops(1):
                op()

    nc.compile()
    return nc


_CACHE = {}


def _ppart(vec, chunks):
    return np.ascontiguousarray(vec.reshape(chunks, P).T, np.float32)


def _pow2scale(x, target=100.0):
    m = float(np.abs(x).max())
    return 2.0 ** math.floor(math.log2(target / m)) if m > 0 else 1.0


def kernel(**inputs):
    f32 = np.float32
    bf16 = ml_dtypes.bfloat16
    e4 = ml_dtypes.float8_e4m3
    x = np.asarray(inputs["x"], f32)
    wv = np.asarray(inputs["wv"], f32)
    bv = np.asarray(inputs["bv"], f32)
    fc_w = np.asarray(inputs["fc_w"], f32)
    fc_b = np.asarray(inputs["fc_b"], f32)
    ln1_g = np.asarray(inputs["ln1_g"], f32)
    ln1_b = np.asarray(inputs["ln1_b"], f32)
    w1 = np.asarray(inputs["w1"], f32)
    b1 = np.asarray(inputs["b1"], f32)
    w2 = np.asarray(inputs["w2"], f32)
    b2 = np.asarray(inputs["b2"], f32)
    ln2_g = np.asarray(inputs["ln2_g"], f32)
    ln2_b = np.asarray(inputs["ln2_b"], f32)
    h2o_w = np.asarray(inputs["h2o_w"], f32)
    h2o_b = np.asarray(inputs["h2o_b"], f32)
    p0 = np.asarray(inputs["p0"], np.float64)
    p1 = np.asarray(inputs["p1"], np.float64)
    p2 = np.asarray(inputs["p2"], np.float64)
    p3 = np.asarray(inputs["p3"], np.float64)
    # wk/bk deliberately unused: constant along the softmax axis.

    sp1 = np.float32(_softplus(p1)).astype(np.float64)
    sp2 = np.float32(_softplus(p2)).astype(np.float64)

    fcb_fold = fc_b.astype(np.float64) + fc_w.astype(np.float64) @ bv.astype(np.float64)
    bias_on = {"h2o": bool(np.any(h2o_b)), "fc": bool(np.any(fcb_fold)),
               "b1": bool(np.any(b1)), "b2": bool(np.any(b2))}
    sw = _pow2scale(h2o_w)
    sw1 = _pow2scale(w1)
    sw2 = _pow2scale(w2)
    sc = {"inv_h2o": 1.0 / (8.0 * sw),
          "kH": 1.0 / (16.0 * sw1),     # psh -> relu(mid) (bf16 H)
          "cY16": 1.0 / 16.0}           # t16 -> Y (residual, unscaled)

    key = (p0.tobytes(), sp1.tobytes(), sp2.tobytes(), p3.tobytes(),
           tuple(sorted(bias_on.items())), sw, sw1, sw2)
    if key not in _CACHE:
        _CACHE[key] = _build(p0, sp1, sp2, p3, bias_on, sc)
    nc = _CACHE[key]

    x_rm = np.ascontiguousarray(
        x.reshape(B, 8, P, D).transpose(2, 1, 0, 3).astype(bf16))
    shared = {
        "w2T": np.ascontiguousarray(w2.T.astype(bf16)),
        "x_rm": x_rm,
        "fcM": np.ascontiguousarray(
            (wv.T.astype(np.float64) @ fc_w.T.astype(np.float64))
            .astype(f32).astype(bf16)),
        "w1T": np.ascontiguousarray((w1.T * sw1).astype(e4)),
        "h2oT": np.ascontiguousarray((h2o_w.T * sw).astype(e4)),
        "onesc": np.ones((P, 2), f32),
        "onesb": np.ones((P, 2), bf16),
        "onesp": np.ones((1, P), f32),
        "ln1g": _ppart(16.0 * ln1_g, DC),
        "ln1b": _ppart(16.0 * ln1_b, DC),
        "ln2g8": _ppart(8.0 * ln2_g, DC),
        "ln2b8": _ppart(8.0 * ln2_b, DC),
    }
    if bias_on["fc"]:
        shared["fcb2"] = _ppart(fcb_fold.astype(f32), DC)
    if bias_on["b1"]:
        shared["b12"] = _ppart(b1, HC)
    if bias_on["b2"]:
        shared["b22"] = _ppart(b2, DC)
    if bias_on["h2o"]:
        shared["h2ob"] = np.ascontiguousarray(
            (h2o_b * 8.0 * sw)[None].astype(bf16))
        shared["onesr"] = np.ones((1, ROWS), bf16)

    j = np.arange(L)
    in_maps = []
    for c in range(NCORES):
        i_idx = c * IC + np.arange(IC)
        Sji = np.abs(j[:, None] - i_idx[None, :]).astype(np.float64)
        posi = p0[0] * (np.exp(-sp1[0] * Sji) + np.exp(-sp2[0] * Sji)) \
            + p3[0] * (i_idx[None, :] < j[:, None])
        u = np.exp(posi)
        u[Sji == 0] = 0.0
        rs = 1.0 / u.sum(axis=0)
        u_t = np.ascontiguousarray(
            u.reshape(8, P, IC).transpose(1, 0, 2).astype(bf16))
        rsb = np.ascontiguousarray(np.broadcast_to(
            np.tile(rs.astype(f32), B)[None, :], (P, ROWS)))

        m = dict(shared)
        m["u_in"] = u_t
        m["rsb_in"] = rsb
        in_maps.append(m)

    res = run_bass_kernel_spmd(nc, in_maps, core_ids=list(range(NCORES)))
    if os.environ.get("K2_DEBUG"):
        global _DBG
        _DBG = res.results

    out = np.empty((B, L, V), f32)
    for c in range(NCORES):
        yc = res.results[c]["y"]
        for b in range(B):
            out[b, c * IC:(c + 1) * IC, :] = yc[b * IC:(b + 1) * IC, :]
    return out
